# revision 5
# baseline (speedup 1.0000x reference)
"""Trainium2 Bass kernel for nn_CCepLTVFilter.

Pipeline (per core; frequency-sharded across 8 cores, FS=128 freqs each):
  1. conv1d(x, W) + b            -> ccep[o, bt]          (PE, fp16 1-pass)
  2. Yr/Yi = DFT of padded ccep  -> [f, bt]              (PE, lhsT = CF/SF)
  3. mag = 10^(Yr/10) via tanh identity; sin/cos(Yi) via ACT Sin
     (single preloaded ACT table #18 covers Copy+Tanh+Sin)
  4. Zr/Zi = 1025-pt DFT of z frames (host-transposed)   (PE, fp16)
  5. Pr + iPi = mag e^{iYi} (Zr + iZi) on DVE:
     av/dv = cos,sin * zr (PSUM direct, overlap zi matmuls),
     bv/cv = sin,cos * zi, u1 = av-bv, u2 = cv+dv, P = rn*u (fp16 2x)
     with rn = (1+tanh)/(1-tanh) * scale
  6. OLA fused into the final DFT: out_b[t, 0:HOP] =
     Pr_b^T CO_l + Prshift_b^T CO_r + Pi_b^T SO_l + Pishift_b^T SO_r (PE)
     where the t-1 circular shift is realized by writing P into a
     [128, 260] tile with per-batch 1-col guard columns.

All matmul inputs are fp16 (m10 keeps rel err ~7e-3 vs the 2e-2 gate;
fp16 halves both DMA bytes and PE passes vs f32r/fp32). CO/SO carry the
Hann window and a x16 rescale (fp16-normal range); 1/(16*1025) is folded
into rn. Inputs are packed into three wide-row DMAs (>=2KB/partition)
so HBM transfers run near peak. Per-core outputs are partial sums of
the full [2,1,32768] output; summed on the host.
"""

import numpy as np

import concourse.bass as bass
import concourse.bacc as bacc
import concourse.mybir as mybir
import concourse.tile as tile
from concourse.bass_utils import run_bass_kernel_spmd

# ---------------- problem dims (hardcoded) ----------------
B, T, D = 2, 128, 80
CCEP = 222
FFT = 1024
HOP = 256
WIN = 2 * HOP            # 512
PAD = (FFT - CCEP) // 2  # 401
M = FFT + 1              # 1025-point transforms
BT = B * T               # 256
NCORES = 8
FS = FFT // NCORES       # 128 frequencies per core
OC = CCEP // 2           # 111 (o-chunk)
LAM = float(np.log(10.0) / 10.0)
COSC = 16.0              # CO/SO rescale; 1/(COSC*M) folded into rn

F32 = mybir.dt.float32
F16 = mybir.dt.float16
PI = float(np.pi)

ACT_TABLE_SIN_TANH = 18  # silu_and_others: covers Copy, Tanh, Sin

TRACE = False            # set by test harness for profiling
LAST_RESULT = None       # BassKernelResults of last run (for test harness)


# ---------------- host-side constants (input independent) ----------------
def _make_constants():
    o = np.arange(CCEP, dtype=np.float64)[:, None]
    f = np.arange(FFT, dtype=np.float64)[None, :]
    qn_idx = np.arange(1, CCEP // 2 + 1, dtype=np.float64)
    qnorm = np.concatenate([qn_idx[::-1], qn_idx])
    ang = 2.0 * np.pi * f * (o + PAD) / FFT
    CF = np.cos(ang) * (LAM / 2.0) / qnorm[:, None]      # [222,1024]
    SF = -np.sin(ang) / qnorm[:, None]

    u = np.arange(WIN, dtype=np.float64)[:, None]
    phi = 2.0 * np.pi * f * (u + FFT // 2) / M
    ZC = np.cos(phi)                                     # [512,1024]
    ZS = np.sin(phi)

    w = np.arange(WIN, dtype=np.float64)[None, :]
    th = 2.0 * np.pi * np.arange(FFT, dtype=np.float64)[:, None] * w / M
    win = 0.5 * (1.0 - np.cos(2.0 * np.pi * np.arange(WIN) / WIN))
    CO = np.cos(th) * win[None, :] * COSC                # [1024,512]
    SO = np.sin(th) * win[None, :] * COSC

    consts = []
    for c in range(NCORES):
        sl = slice(c * FS, (c + 1) * FS)
        # cf/sf [111, 256] each, cols (oc, f) -> packed into spk on host
        cfp = CF[:, sl].reshape(2, OC, FS).transpose(1, 0, 2).reshape(OC, 2 * FS)
        sfp = SF[:, sl].reshape(2, OC, FS).transpose(1, 0, 2).reshape(OC, 2 * FS)
        cp1 = np.zeros((128, 4 * FS), np.float32)
        cp1[0:OC, 0:2 * FS] = cfp
        cp1[0:OC, 2 * FS:4 * FS] = sfp
        # zczs [128, 1024]: zc(u-chunk mc, f) | zs
        zcp = ZC[:, sl].reshape(4, 128, FS).transpose(1, 0, 2).reshape(128, 4 * FS)
        zsp = ZS[:, sl].reshape(4, 128, FS).transpose(1, 0, 2).reshape(128, 4 * FS)
        zczs = np.concatenate([zcp, zsp], axis=1)
        # cp3 [128, 1024] = [co_l | co_r | so_l | so_r]
        cob, sob = CO[sl, :], SO[sl, :]
        cp3 = np.concatenate(
            [cob[:, :HOP], cob[:, HOP:], sob[:, :HOP], sob[:, HOP:]], axis=1)
        consts.append(dict(cp1=cp1.astype(np.float16),
                           zczs=zczs.astype(np.float16),
                           cp3=cp3.astype(np.float16)))
    return consts


_CONSTS = _make_constants()
_NC = None


# ---------------- device program ----------------
def _build_nc():
    nc = bacc.Bacc()
    # spk = spack (xcat+w2, 956 cols) | cp1 (cf+sf, 512 cols): 2936B rows
    sp_e = nc.dram_tensor("spk", [128, 1468], F16, kind="ExternalInput")
    fr_e = nc.dram_tensor("frp", [128, 4 * BT], F16, kind="ExternalInput")
    zz_e = nc.dram_tensor("zczs", [128, 8 * FS], F16, kind="ExternalInput")
    c3_e = nc.dram_tensor("cp3", [128, 8 * FS], F16, kind="ExternalInput")
    out_e = nc.dram_tensor("out", [B, 1, T * HOP], F32, kind="ExternalOutput")

    with tile.TileContext(nc) as tc:
        with tc.tile_pool(name="sb", bufs=1) as sb, \
             tc.tile_pool(name="ps", bufs=1, space="PSUM") as ps:

            # ---- input DMAs; queue = issuing engine ----
            spk = sb.tile([128, 1468], F16, tag="spk", name="spk")
            nc.scalar.dma_start(out=spk[:], in_=sp_e[:, :])
            frp = sb.tile([128, 4 * BT], F16, tag="frp", name="frp")
            nc.sync.dma_start(out=frp[:], in_=fr_e[:, :])
            zczs = sb.tile([128, 8 * FS], F16, tag="zczs", name="zczs")
            nc.gpsimd.dma_start(out=zczs[:], in_=zz_e[:, :])
            cp3 = sb.tile([128, 8 * FS], F16, tag="cp3", name="cp3")
            nc.gpsimd.dma_start(out=cp3[:], in_=c3_e[:, :])

            xcatA = spk[0:121, 0:BT]
            xcatB = spk[0:120, BT:2 * BT]
            w2A = spk[0:121, 2 * BT:2 * BT + CCEP]
            w2B = spk[0:120, 2 * BT + CCEP:2 * BT + 2 * CCEP]
            cf = spk[0:OC, 956:956 + 2 * FS]
            sf = spk[0:OC, 956 + 2 * FS:956 + 4 * FS]
            zc = zczs[:, 0:4 * FS]
            zs = zczs[:, 4 * FS:8 * FS]
            co_l = cp3[:, 0:HOP]
            co_r = cp3[:, HOP:2 * HOP]
            so_l = cp3[:, 2 * HOP:3 * HOP]
            so_r = cp3[:, 3 * HOP:4 * HOP]

            # ---- conv: ccep[o, bt] = W2.T @ xcat (bias via ones row) ----
            ccep = []
            for c in range(2):
                pc = ps.tile([OC, BT], F32, tag=f"conv{c}", name=f"conv{c}")
                nc.tensor.matmul(pc[:, :], w2A[:, c * OC:(c + 1) * OC],
                                 xcatA, start=True, stop=False)
                nc.tensor.matmul(pc[:, :], w2B[:, c * OC:(c + 1) * OC],
                                 xcatB, start=False, stop=True)
                cs = sb.tile([OC, BT], F16, tag=f"ccep{c}", name=f"ccep{c}")
                nc.vector.tensor_copy(cs[:, :], pc[:, :])
                ccep.append(cs)

            # ---- step2: Yr/Yi [f_local, bt] ----
            yr = ps.tile([FS, BT], F32, tag="yr", name="yr")
            yi = ps.tile([FS, BT], F32, tag="yi", name="yi")
            for c in range(2):
                nc.tensor.matmul(yr[:, :], cf[:, c * FS:(c + 1) * FS],
                                 ccep[c][:, :], start=(c == 0), stop=(c == 1))
            for c in range(2):
                nc.tensor.matmul(yi[:, :], sf[:, c * FS:(c + 1) * FS],
                                 ccep[c][:, :], start=(c == 0), stop=(c == 1))

            # ---- step3 ----
            def wtile(name, dt=F32):
                return sb.tile([FS, BT], dt, tag=name, name=name)

            # ACT: tanh first (feeds den/num), then sin/cos (fp16 out)
            th = wtile("th")
            nc.scalar.activation(th[:, :], yr[:, :],
                                 mybir.ActivationFunctionType.Tanh)
            yiw = wtile("yiw")
            nc.vector.add_range_wrap(yiw[:, :], yi[:, :], 0.0, PI, 2.0 * PI)
            yic = wtile("yic")
            nc.vector.add_range_wrap(yic[:, :], yi[:, :], PI / 2.0, PI, 2.0 * PI)
            sinv = wtile("sinv", F16)
            nc.scalar.activation(sinv[:, :], yiw[:, :],
                                 mybir.ActivationFunctionType.Sin)
            cosv = wtile("cosv", F16)
            nc.scalar.activation(cosv[:, :], yic[:, :],
                                 mybir.ActivationFunctionType.Sin)
            # rn = (1+t)/(1-t) / (M*COSC), t = tanh(Yr * ln10/20)
            den = wtile("den")
            nc.vector.tensor_scalar(den[:, :], th[:, :], -1.0, 1.0,
                                    mybir.AluOpType.mult, mybir.AluOpType.add)
            rscr = wtile("rscr")
            rcp = wtile("rcp")
            nc.vector.reciprocal_approx_accurate(rcp[:, :], den[:, :], rscr[:, :])
            s = 1.0 / (M * COSC)
            num = wtile("num")
            nc.vector.tensor_scalar(num[:, :], th[:, :], s, s,
                                    mybir.AluOpType.mult, mybir.AluOpType.add)
            rn = wtile("rn", F16)
            nc.vector.tensor_tensor(rn[:, :], num[:, :], rcp[:, :],
                                    mybir.AluOpType.mult)

            # ---- step4: Zr/Zi [f_local, bt] ----
            zr = ps.tile([FS, BT], F32, tag="zr", name="zr")
            zi = ps.tile([FS, BT], F32, tag="zi", name="zi")
            for mc in range(4):
                nc.tensor.matmul(zr[:, :], zc[:, mc * FS:(mc + 1) * FS],
                                 frp[:, mc * BT:(mc + 1) * BT],
                                 start=(mc == 0), stop=(mc == 3))
            for mc in range(4):
                nc.tensor.matmul(zi[:, :], zs[:, mc * FS:(mc + 1) * FS],
                                 frp[:, mc * BT:(mc + 1) * BT],
                                 start=(mc == 0), stop=(mc == 3))

            # ---- step5 (DVE): P = rn (cos + i sin)(zr + i zi) ----
            # P tiles [128, 260]: [g0 | P_b0 (128) | g1 | P_b1 (128) | pad]
            # guard col g_b = P_b[t=127] -> window [g_b..] is P_b[(t-1)%T].
            prx = sb.tile([FS, 260], F16, tag="prx", name="prx")
            pix = sb.tile([FS, 260], F16, tag="pix", name="pix")

            def p_main_ap(t):
                full = t[:, :]
                return bass.AP(full.tensor, full.offset + 1,
                               [full.ap[0], [129, 2], [1, T]])

            def p_wrap(t):
                full = t[:, :]
                dst = bass.AP(full.tensor, full.offset,
                              [full.ap[0], [129, 2], [1, 1]])
                src = bass.AP(full.tensor, full.offset + T,
                              [full.ap[0], [129, 2], [1, 1]])
                return dst, src

            # av/dv read zr from PSUM and overlap the zi matmuls
            av = wtile("av", F16)
            nc.vector.tensor_tensor(av[:, :], cosv[:, :], zr[:, :],
                                    mybir.AluOpType.mult)
            dv = wtile("dv", F16)
            nc.vector.tensor_tensor(dv[:, :], sinv[:, :], zr[:, :],
                                    mybir.AluOpType.mult)
            bv = wtile("bv", F16)
            nc.vector.tensor_tensor(bv[:, :], sinv[:, :], zi[:, :],
                                    mybir.AluOpType.mult)
            u1 = wtile("u1", F16)
            nc.vector.tensor_tensor(u1[:, :], av[:, :], bv[:, :],
                                    mybir.AluOpType.subtract)
            nc.vector.tensor_tensor(p_main_ap(prx), rn[:, :], u1[:, :],
                                    mybir.AluOpType.mult)
            dst, src = p_wrap(prx)
            nc.scalar.copy(dst, src)

            cv = wtile("cv", F16)
            nc.vector.tensor_tensor(cv[:, :], cosv[:, :], zi[:, :],
                                    mybir.AluOpType.mult)
            u2 = wtile("u2", F16)
            nc.vector.tensor_tensor(u2[:, :], cv[:, :], dv[:, :],
                                    mybir.AluOpType.add)
            nc.vector.tensor_tensor(p_main_ap(pix), rn[:, :], u2[:, :],
                                    mybir.AluOpType.mult)
            dst, src = p_wrap(pix)
            nc.scalar.copy(dst, src)

            # ---- step6 + OLA (fused): per b, out_ob[t, j] =
            #  Pr_b^T co_l + Prs_b^T co_r + Pi_b^T so_l + Pis_b^T so_r
            for bb in range(B):
                o0 = 1 + bb * 129
                pr_b = prx[:, o0:o0 + T]
                prs_b = prx[:, o0 - 1:o0 - 1 + T]
                pi_b = pix[:, o0:o0 + T]
                pis_b = pix[:, o0 - 1:o0 - 1 + T]
                ob = ps.tile([T, HOP], F32, tag=f"ob{bb}", name=f"ob{bb}")
                nc.tensor.matmul(ob[:, :], pr_b, co_l, start=True, stop=False)
                nc.tensor.matmul(ob[:, :], prs_b, co_r, start=False, stop=False)
                nc.tensor.matmul(ob[:, :], pi_b, so_l, start=False, stop=False)
                nc.tensor.matmul(ob[:, :], pis_b, so_r, start=False, stop=True)
                obs = sb.tile([T, HOP], F32, tag=f"obs{bb}", name=f"obs{bb}")
                if bb == 0:
                    nc.scalar.copy(obs[:, :], ob[:, :])
                else:
                    nc.vector.tensor_copy(obs[:, :], ob[:, :])
                dst = bass.AP(out_e[:, :, :].tensor, bb * T * HOP,
                              [[HOP, T], [1, HOP]])
                nc.sync.dma_start(out=dst, in_=obs[:, :])

    return nc


def _patch_act_table(nc):
    """Pre-place a single ACT table load (table 18 covers Copy+Tanh+Sin)
    instead of the default pass's two loads (exp table then trig table).
    Placed just before the first InstActivation so the scalar engine's
    DMA issues at the top of the block are not delayed by the load."""
    def my_insert():
        for b in nc.main_func.blocks:
            idx = None
            for j, i in enumerate(b.instructions):
                if isinstance(i, mybir.InstActivation):
                    idx = j
                    break
            if idx is None:
                continue
            ld = mybir.InstLoadActFuncSet(
                name=nc.get_next_instruction_name(),
                act_func_set_id=ACT_TABLE_SIN_TANH, ins=[], outs=[])
            ld.engine = mybir.EngineType.Activation
            nc.register_instruction(ld)
            b.instructions.insert(idx, ld)
            return
    nc.insert_act_table_loads = my_insert


def _get_nc():
    global _NC
    if _NC is None:
        _NC = _build_nc()
        _patch_act_table(_NC)
        _NC.finalize()
    return _NC


# ---------------- host orchestration ----------------
def _prep_inputs(x, z, W, b, cp1):
    x = np.ascontiguousarray(np.asarray(x, dtype=np.float32))
    z = np.ascontiguousarray(np.asarray(z, dtype=np.float32))
    W = np.ascontiguousarray(np.asarray(W, dtype=np.float32))
    b = np.ascontiguousarray(np.asarray(b, dtype=np.float32))

    xT = np.ascontiguousarray(x.reshape(BT, D).T)                 # [80, 256]
    xsh = np.zeros((3, D, BT), np.float32)
    xsh[1] = xT
    xv = xT.reshape(D, B, T)
    xsh[0].reshape(D, B, T)[:, :, 1:] = xv[:, :, :-1]
    xsh[2].reshape(D, B, T)[:, :, :-1] = xv[:, :, 1:]
    xcat = np.concatenate([xsh.reshape(3 * D, BT),
                           np.ones((1, BT), np.float32)], axis=0)  # [241,256]
    w2 = np.concatenate([W[:, :, 0].T, W[:, :, 1].T, W[:, :, 2].T,
                         b[None, :]], axis=0)                      # [241,222]
    spk = np.zeros((128, 1468), np.float16)
    spk[0:121, 0:BT] = xcat[0:121].astype(np.float16)
    spk[0:120, BT:2 * BT] = xcat[121:241].astype(np.float16)
    spk[0:121, 2 * BT:2 * BT + CCEP] = w2[0:121].astype(np.float16)
    spk[0:120, 2 * BT + CCEP:2 * BT + 2 * CCEP] = w2[121:241].astype(np.float16)
    spk[:, 956:1468] = cp1

    zpad = np.concatenate(
        [np.zeros((B, HOP), np.float32), z[:, 0, :]], axis=1)     # [2, 33024]
    frames = np.lib.stride_tricks.sliding_window_view(
        zpad, WIN, axis=1)[:, ::HOP][:, :T]                       # [B, T, WIN]
    frp = frames.transpose(2, 0, 1).reshape(4, 128, B, T) \
        .transpose(1, 0, 2, 3).reshape(128, 4 * BT)               # [128, 1024]

    return spk, np.ascontiguousarray(frp).astype(np.float16)


def kernel(x, z, W, b):
    global LAST_RESULT
    in_maps = []
    frp = None
    for c in range(NCORES):
        cst = _CONSTS[c]
        if frp is None:
            spk0, frp = _prep_inputs(x, z, W, b, cst["cp1"])
            spk = spk0
        else:
            spk = spk0.copy()
            spk[:, 956:1468] = cst["cp1"]
        in_maps.append({"spk": spk, "frp": frp,
                        "zczs": cst["zczs"], "cp3": cst["cp3"]})

    nc = _get_nc()
    res = run_bass_kernel_spmd(nc, in_maps, list(range(NCORES)), trace=TRACE)
    LAST_RESULT = res
    out = np.zeros((B, 1, T * HOP), dtype=np.float32)
    for r in res.results:
        out += np.asarray(r["out"], dtype=np.float32)
    return out
